# revision 34
# baseline (speedup 1.0000x reference)
"""Trainium2 Bass kernel: LocalWLGNN 3-hop GNN message passing on 8 NeuronCores.

Strategy (dst-node sharding):
  - out = (1+eps)*x + sum_h h_h, with per-hop recurrence
        h_new = a1 * G + w (.) x + c1,   G[r] = sum_{e: ni[e]=r} h[si[e]]
    where a1=(1+b1)(1+b3), c1=(1+b3), w=(1+b3)(deg + untouched + b2) are
    host-folded from the runtime scalar inputs (exact algebra, verified).
  - Nodes are dst-sharded across 8 cores (12500 rows each, padded to 12544).
    Each core computes G for its shard via dma_gather (random source rows,
    bf16) + per-128-edge-tile selection-matrix matmul into PSUM block
    accumulators (128 dst rows per block), then fused elementwise tails.
  - dma_gather calls round-robin SWDGE queue_num 0-3 (num_swdge_queues=4) so
    the Q7 descriptor generation runs on all four core pairs concurrently
    (~3.5x over a single queue; descgen is the kernel's critical resource).
  - dma_gather indices are int16, so sources live in 4 address-range buckets.
    Buckets coincide with "quarters" of a padded-global layout that
    interleaves per-core quarter slices ([q: core0 rows, core1 rows, ...]),
    letting each quarter be one AllGather into its own Shared DRAM tensor.
    The 4 per-hop collectives fire as soon as the corresponding quarter of
    h is computed, overlapping the halo exchange with remaining compute.
  - Edge schedule is uniform across cores: per (bucket, block) segments of
    length max-over-cores, concatenated per (superblock, bucket) with
    tiles SHARED across adjacent segments (no per-group ceil-to-128
    padding); off=200 masks foreign-block lanes in each (block, tile)
    selection matmul. Pad gathers read row 0 of the bucket.
  - Selection matrices for all tiles of a block are built in one fused
    3-dim is_equal; elementwise tails are fused across each superblock.
    The h intermediate (tmp2) is bf16: h stores DMA straight from it and
    DVE runs the bulk elementwise at 16-bit rate. Final out rows are
    written back per superblock to spread the output DMA over hop 2.
"""

import sys

sys.path.insert(0, "/opt/trn_rl_repo")

import numpy as np
import ml_dtypes

from concourse import bass, bacc, mybir
import concourse.tile as tile
from concourse.bass_utils import run_bass_kernel_spmd

P = 128
D = 128
HOPS = 3

FULL_CFG = dict(
    N=100000,
    NCORES=8,
    S=12500,        # rows per core
    NB=98,          # dst blocks per core (ceil(S/128)); SPAD = NB*128 = 12544
    SB_BLOCKS=9,    # blocks per superblock (PSUM-resident group)
    NBUCKETS=4,     # source-range buckets (NPAD/NBUCKETS must be < 32768)
)


def _derived(cfg):
    S, NB, NC = cfg["S"], cfg["NB"], cfg["NCORES"]
    SPAD = NB * P
    NPAD = NC * SPAD
    BUCKET = NPAD // cfg["NBUCKETS"]
    assert NPAD % cfg["NBUCKETS"] == 0 and BUCKET % 2 == 0 and BUCKET <= 32767
    return SPAD, NPAD, BUCKET


Q_BLOCKS = [25, 25, 24, 24]          # per-core blocks per quarter
Q_LO = [0, 3200, 6400, 9472]         # per-core row offset of each quarter
Q_ROWS = [3200, 3200, 3072, 3072]
Q_LAST_BLOCK = [24, 49, 73, 97]


def _pos_map(si, cfg):
    """Padded-global position with quarter-interleaved layout:
    [q0: core0 rows, core1 rows, ...][q1: ...]. Each quarter is one
    AllGather slice and one gather bucket."""
    S, NC = cfg["S"], cfg["NCORES"]
    c = si // S
    r = si % S
    q = np.searchsorted(np.array([25, 50, 74]), r >> 7, side="right")
    qlo = np.take(np.array(Q_LO, np.int64), q)
    qrows = np.take(np.array(Q_ROWS, np.int64), q)
    return NC * qlo + c * qrows + (r - qlo)


def _wrap_idx(a):
    """Gather-index SBUF layout: logical position i -> [partition i%16, free i//16],
    replicated across the 8 q7 cores (x8 on partitions)."""
    assert len(a) % 128 == 0
    w = a.reshape(-1, 16).T.astype(np.int16)
    return np.tile(w, (8, 1))


def _plan_hop(ni_loc_list, si_pad_list, cfg):
    """Build a per-hop schedule, uniform across cores.

    ni_loc_list[c]: local dst rows in [0, S); si_pad_list[c]: padded-global src rows.
    Returns (sched, percore) where
      sched = dict(
        n_tiles, tot_idx,
        sbs = [ dict(gathers={b: (idx_free_off16, L)},
                     blocks=[ (jg, [ (b, vslot, gtile), ... ]), ... ]) ])
      percore[c] = dict(idx=[128, tot_idx//16] int16, off=[128, n_tiles] f32)
    """
    NB, NBUK, SBB, NC = cfg["NB"], cfg["NBUCKETS"], cfg["SB_BLOCKS"], cfg["NCORES"]
    NSB = (NB + SBB - 1) // SBB
    gb_start = np.cumsum([0] + [NC * r for r in Q_ROWS])[:NBUK].astype(np.int64)

    counts = np.zeros((NC, NBUK, NB), np.int64)
    sorted_src = []
    sorted_off = []
    group_start = []
    for c in range(NC):
        ni, si = ni_loc_list[c], si_pad_list[c]
        blk = ni >> 7
        buk = np.searchsorted(gb_start, si, side="right") - 1
        key = (buk.astype(np.int64) * NB + blk) * (1 << 17) + si
        order = np.argsort(key, kind="stable")
        sorted_src.append(si[order])
        sorted_off.append((ni & 127).astype(np.float32)[order])
        cnt = np.bincount(buk.astype(np.int64) * NB + blk,
                          minlength=NBUK * NB).reshape(NBUK, NB)
        counts[c] = cnt
        gs = np.zeros(NBUK * NB + 1, np.int64)
        np.cumsum(cnt.reshape(-1), out=gs[1:])
        group_start.append(gs)

    cnt_u = counts.max(axis=0)  # [NBUK, NB] uniform segment lengths

    # stream layout: per (sb, b) chunk, segments of cnt_u[b, j] for j in sb,
    # padded at the chunk end to a multiple of 128. Tiles are shared across
    # adjacent segments; off=200 masks foreign-block lanes per (block, tile)
    # matmul pair. Pair ids are allocated in (sb, j, b, t) order so each
    # block's pairs are consecutive.
    chunk_off = {}    # (sbi, b) -> (stream_offset, L)
    seg_start = np.zeros((NBUK, NB), np.int64)  # global stream pos of segment
    stream_pos = 0
    for sbi in range(NSB):
        for b in range(NBUK):
            start = stream_pos
            for j in range(sbi * SBB, min((sbi + 1) * SBB, NB)):
                seg_start[b, j] = stream_pos
                stream_pos += int(cnt_u[b, j])
            L = -(-(stream_pos - start) // 128) * 128
            stream_pos = start + L
            chunk_off[(sbi, b)] = (start, L)
    tot_idx = stream_pos
    assert tot_idx % 128 == 0 and tot_idx > 0

    # pairs + schedule for the device builder
    sbs = []
    pair_list = []  # (b, j, tile_in_chunk, lane_lo, lane_hi) per pair id
    p_ctr = 0
    for sbi in range(NSB):
        blocks = []
        for j in range(sbi * SBB, min((sbi + 1) * SBB, NB)):
            tiles = []
            for b in range(NBUK):
                cu = int(cnt_u[b, j])
                if cu == 0:
                    continue
                c0 = chunk_off[(sbi, b)][0]
                s0 = int(seg_start[b, j]) - c0
                t0, t1 = s0 // 128, (s0 + cu - 1) // 128
                for t in range(t0, t1 + 1):
                    tiles.append((b, t, p_ctr))
                    pair_list.append((b, j, t))
                    p_ctr += 1
            blocks.append((j, tiles))
        gathers = {b: chunk_off[(sbi, b)] for b in range(NBUK)
                   if chunk_off[(sbi, b)][1] > 0}
        sbs.append(dict(gathers=gathers, blocks=blocks))
    n_pairs = p_ctr

    percore = []
    for c in range(NC):
        idx_stream = np.zeros(tot_idx, np.int32)
        grows = np.zeros(tot_idx, np.int64)
        off_flat = np.full((n_pairs, 128), 200.0, np.float32)
        gs = group_start[c]
        ss, so = sorted_src[c], sorted_off[c]
        for b in range(NBUK):
            for j in range(NB):
                cnt = int(counts[c, b, j])
                if cnt == 0:
                    continue
                g0 = gs[b * NB + j]
                pos = int(seg_start[b, j])
                idx_stream[pos:pos + cnt] = ss[g0:g0 + cnt] - gb_start[b]
                grows[pos:pos + cnt] = ss[g0:g0 + cnt]
        for pid, (b, j, t) in enumerate(pair_list):
            sbi = j // SBB
            c0 = chunk_off[(sbi, b)][0]
            s0 = int(seg_start[b, j])
            cnt = int(counts[c, b, j])
            # lanes of tile t (global stream [c0+128t, c0+128t+128)) that hold
            # this core's (b, j) edges
            lo = max(c0 + 128 * t, s0)
            hi = min(c0 + 128 * (t + 1), s0 + cnt)
            if lo < hi:
                g0 = gs[b * NB + j]
                off_flat[pid, lo - c0 - 128 * t: hi - c0 - 128 * t] = \
                    so[g0 + lo - s0: g0 + hi - s0]
        # wrap idx per (sb, b) chunk
        idx_w = np.zeros((128, tot_idx // 16), np.int16)
        for (sbi, b), (start, L) in chunk_off.items():
            if L > 0:
                idx_w[:, start // 16: (start + L) // 16] = _wrap_idx(
                    idx_stream[start:start + L].astype(np.int16))
        percore.append(dict(idx=idx_w, grows=grows,
                            off=np.ascontiguousarray(off_flat.T).astype(ml_dtypes.bfloat16)))

    sched = dict(n_tiles=n_pairs, tot_idx=tot_idx, sbs=sbs)
    return sched, percore


def _build_nc(cfg, scheds, scalars, hops=HOPS):
    """Build the SPMD bass program. scheds: per-hop schedule; scalars: dict with
    eps, a1[h], c1[h] floats baked as immediates."""
    NB, NBUK, NC = cfg["NB"], cfg["NBUCKETS"], cfg["NCORES"]
    SPAD, NPAD, BUCKET = _derived(cfg)
    f32, bf16, i16 = mybir.dt.float32, mybir.dt.bfloat16, mybir.dt.int16
    AOP = mybir.AluOpType

    nc = bacc.Bacc("TRN2", target_bir_lowering=False, debug=False, num_devices=NC,
                   num_swdge_queues=4)

    xg = nc.dram_tensor("xg", [NPAD, D], bf16, kind="ExternalInput")
    xres_d = nc.dram_tensor("xres", [P, NB, D], f32, kind="ExternalInput")
    wres_d = nc.dram_tensor("wres", [P, HOPS * NB], f32, kind="ExternalInput")
    iota_d = nc.dram_tensor("iota", [P, P], bf16, kind="ExternalInput")
    cvec_d = nc.dram_tensor("cvec", [P, HOPS + 1], f32, kind="ExternalInput")
    idx_d = [nc.dram_tensor(f"idx{h}", [P, scheds[h]["tot_idx"] // 16], i16,
                            kind="ExternalInput") for h in range(HOPS)]
    off_d = [nc.dram_tensor(f"off{h}", [P, scheds[h]["n_tiles"]], bf16,
                            kind="ExternalInput") for h in range(HOPS)]
    out_d = nc.dram_tensor("out", [P, NB, D], f32, kind="ExternalOutput")

    eps = scalars["eps"]
    a1 = scalars["a1"]
    c1 = scalars["c1"]

    with tile.TileContext(nc) as tc:
        with (
            tc.tile_pool(name="const", bufs=1) as cpool,
            tc.tile_pool(name="io", bufs=2) as iopool,
            tc.tile_pool(name="v", bufs=3) as vpool,
            tc.tile_pool(name="m", bufs=3) as mpool,
            tc.tile_pool(name="t2", bufs=2) as t2pool,
            tc.tile_pool(name="fin", bufs=4) as fpool,
            tc.tile_pool(name="ps", bufs=6, space="PSUM") as pspool,
            tc.tile_pool(name="dram", bufs=1, space="DRAM") as dpool,
        ):
            iota_t = cpool.tile([P, P], bf16, name="iota_t")
            nc.sync.dma_start(out=iota_t[:], in_=iota_d[:])
            cvec = cpool.tile([P, HOPS + 1], f32, name="cvec_t")
            nc.sync.dma_start(out=cvec[:], in_=cvec_d[:])
            xres = cpool.tile([P, NB, D], f32, name="xres_t")
            nc.scalar.dma_start(out=xres[:], in_=xres_d[:])
            wres = cpool.tile([P, HOPS * NB], f32, name="wres_t")
            nc.scalar.dma_start(out=wres[:], in_=wres_d[:])
            out_acc = cpool.tile([P, NB, D], f32, name="out_acc")

            h_my = [dpool.tile([SPAD, D], bf16, name=f"h_my{h}")
                    for h in range(HOPS - 1)]
            h_full = [[dpool.tile([NC * Q_ROWS[q], D], bf16, addr_space="Shared",
                                  name=f"h_full{h}_{q}") for q in range(4)]
                      for h in range(HOPS - 1)]

            self_rr = [0]
            for hop in range(hops):
                sched = scheds[hop]
                gbs = np.cumsum([0] + [NC * r for r in Q_ROWS])
                tables = ([xg[int(gbs[b]):int(gbs[b + 1]), :] for b in range(4)]
                          if hop == 0 else
                          [h_full[hop - 1][b][:] for b in range(4)])
                idx_t = iopool.tile([P, sched["tot_idx"] // 16], i16, tag="idx")
                nc.sync.dma_start(out=idx_t[:], in_=idx_d[hop][:])
                off_t = iopool.tile([P, sched["n_tiles"]], bf16, tag="off")
                nc.sync.dma_start(out=off_t[:], in_=off_d[hop][:])

                for sbi, sb in enumerate(sched["sbs"]):
                    vts = {}
                    for b, (start, L) in sb["gathers"].items():
                        vt = vpool.tile([P, L // 128, D], bf16, tag=f"v{b}")
                        for o in range(0, L, 1024):
                            Lc = min(1024, L - o)
                            nc.gpsimd.dma_gather(
                                vt[:, o // 128:(o + Lc) // 128, :],
                                tables[b],
                                idx_t[:, (start + o) // 16:(start + o + Lc) // 16],
                                Lc, Lc, D,
                                queue_num=self_rr[0] % 4,
                            )
                            self_rr[0] += 1
                        vts[b] = vt
                    jg0 = sb["blocks"][0][0]
                    nsbb = len(sb["blocks"])
                    # tmp2 = w * x + c1 for the whole superblock (fused)
                    tmp2 = t2pool.tile([P, nsbb, D], bf16, name="tmp2")
                    nc.vector.tensor_tensor(
                        out=tmp2[:],
                        in0=wres[:, hop * NB + jg0: hop * NB + jg0 + nsbb]
                            .to_broadcast([P, nsbb, D]),
                        in1=xres[:, jg0:jg0 + nsbb, :], op=AOP.mult)
                    nc.vector.tensor_tensor(
                        out=tmp2[:],
                        in0=cvec[:, hop:hop + 1].unsqueeze(1)
                            .broadcast_to([P, nsbb, D]),
                        in1=tmp2[:], op=AOP.add)
                    for jg, tiles in sb["blocks"]:
                        ntj = len(tiles)
                        if ntj:
                            # tiles of one block have consecutive global ids
                            gt0 = tiles[0][2]
                            assert [t[2] for t in tiles] == list(range(gt0, gt0 + ntj))
                            Mt = mpool.tile([P, ntj, P], bf16, name="Mt")
                            nc.vector.tensor_tensor(
                                out=Mt[:],
                                in0=off_t[:, gt0:gt0 + ntj].to_broadcast([P, ntj, P]),
                                in1=iota_t[:].unsqueeze(1).broadcast_to([P, ntj, P]),
                                op=AOP.is_equal,
                            )
                            ps = pspool.tile([P, D], f32, name="ps")
                            for k, (b, vslot, gt) in enumerate(tiles):
                                nc.tensor.matmul(
                                    out=ps[:],
                                    lhsT=Mt[:, k, :],
                                    rhs=vts[b][:, vslot, :],
                                    start=(k == 0),
                                    stop=(k == ntj - 1),
                                )
                            if a1[hop] != 1.0:
                                nc.vector.tensor_scalar(
                                    out=ps[:], in0=ps[:], scalar1=float(a1[hop]),
                                    scalar2=None, op0=AOP.mult,
                                )
                            # h = a1*G + w*x + c1, in place
                            nc.vector.tensor_tensor(
                                out=tmp2[:, jg - jg0, :],
                                in0=ps[:], in1=tmp2[:, jg - jg0, :], op=AOP.add)
                        if hop < hops - 1:
                            nc.scalar.dma_start(
                                out=h_my[hop][jg * P:(jg + 1) * P, :],
                                in_=tmp2[:, jg - jg0, :])
                    oblk = out_acc[:, jg0:jg0 + nsbb, :]
                    if hop == 0:
                        nc.vector.tensor_tensor(
                            out=oblk,
                            in0=cvec[:, HOPS:HOPS + 1].unsqueeze(1)
                                .broadcast_to([P, nsbb, D]),
                            in1=xres[:, jg0:jg0 + nsbb, :], op=AOP.mult)
                    nc.vector.tensor_tensor(
                        out=oblk, in0=oblk, in1=tmp2[:], op=AOP.add)
                    if hop == hops - 1:
                        nc.sync.dma_start(out=out_d[:, jg0:jg0 + nsbb, :],
                                          in_=oblk)
                    if hop < hops - 1:
                        for q in range(4):
                            if jg0 + nsbb - 1 >= Q_LAST_BLOCK[q] > jg0 + nsbb - 1 - nsbb:
                                nc.gpsimd.collective_compute(
                                    "AllGather",
                                    mybir.AluOpType.bypass,
                                    replica_groups=[list(range(NC))],
                                    ins=[h_my[hop][Q_LO[q]:Q_LO[q] + Q_ROWS[q], :].opt()],
                                    outs=[h_full[hop][q][:].opt()],
                                )
    nc.compile()
    return nc


def _prepare(x, eps, b1, b2, b3, si_list, ni_list, cfg):
    """Host-side folding + sharding. Returns (scheds, scalars, in_maps)."""
    N, NC, S, NB = cfg["N"], cfg["NCORES"], cfg["S"], cfg["NB"]
    SPAD, NPAD, _ = _derived(cfg)

    scalars = dict(
        eps=float(eps),
        a1=[float((1.0 + b1[h]) * (1.0 + b3[h])) for h in range(HOPS)],
        c1=[float(1.0 + b3[h]) for h in range(HOPS)],
    )

    # padded-coordinate gather table of x (bf16), shared by all cores,
    # in the sb-interleaved layout
    xg = np.zeros((NPAD, D), ml_dtypes.bfloat16)
    nodes = np.arange(N, dtype=np.int64)
    xg[_pos_map(nodes, cfg)] = x

    iota = np.tile(np.arange(P, dtype=np.float32), (P, 1)).astype(ml_dtypes.bfloat16)

    # per-core resident x and w
    xres_list, w_all = [], []
    for h in range(HOPS):
        deg = np.bincount(ni_list[h], minlength=N).astype(np.float32)
        untouched = (deg == 0).astype(np.float32)
        w = (1.0 + float(b3[h])) * (deg + untouched + float(b2[h]))
        w_all.append(w)
    for c in range(NC):
        xs = np.zeros((SPAD, D), np.float32)
        lo, hi = c * S, min((c + 1) * S, N)
        xs[: hi - lo] = x[lo:hi]
        xres_list.append(
            np.ascontiguousarray(
                xs.reshape(NB, P, D).transpose(1, 0, 2)))
    wres_list = []
    for c in range(NC):
        ws = np.zeros((HOPS, SPAD), np.float32)
        lo, hi = c * S, min((c + 1) * S, N)
        for h in range(HOPS):
            ws[h, : hi - lo] = w_all[h][lo:hi]
        wres_list.append(
            np.ascontiguousarray(
                ws.reshape(HOPS, NB, P).transpose(2, 0, 1).reshape(P, HOPS * NB)))

    scheds, idx_np, off_np = [], [], []
    grows_np = None
    for h in range(HOPS):
        si, ni = si_list[h], ni_list[h]
        si_pad = _pos_map(si, cfg)
        ni_core = ni // S
        ni_locs, si_pads = [], []
        for c in range(NC):
            m = ni_core == c
            ni_locs.append((ni[m] - c * S).astype(np.int64))
            si_pads.append(si_pad[m].astype(np.int64))
        sched, percore = _plan_hop(ni_locs, si_pads, cfg)
        scheds.append(sched)
        idx_np.append([pc["idx"] for pc in percore])
        off_np.append([pc["off"] for pc in percore])
        if h == 0:
            grows_np = [pc["grows"] for pc in percore]

    in_maps = []
    for c in range(NC):
        cvec = np.tile(np.array(scalars['c1'] + [1.0 + scalars['eps']], np.float32), (P, 1))
        m = dict(xg=xg, xres=xres_list[c], wres=wres_list[c], iota=iota, cvec=cvec)
        for h in range(HOPS):
            m[f"idx{h}"] = idx_np[h][c]
            m[f"off{h}"] = off_np[h][c]
        in_maps.append(m)
    return scheds, scalars, in_maps


def run(x, eps, b1, b2, b3, si_list, ni_list, cfg, trace=False, hops=HOPS):
    scheds, scalars, in_maps = _prepare(x, eps, b1, b2, b3, si_list, ni_list, cfg)
    nc = _build_nc(cfg, scheds, scalars, hops=hops)
    res = run_bass_kernel_spmd(nc, in_maps, list(range(cfg["NCORES"])), trace=trace)
    N, NC, S, NB = cfg["N"], cfg["NCORES"], cfg["S"], cfg["NB"]
    SPAD = NB * P
    parts = []
    for c in range(NC):
        o = res.results[c]["out"].reshape(P, NB, D).transpose(1, 0, 2).reshape(SPAD, D)
        lo, hi = c * S, min((c + 1) * S, N)
        parts.append(o[: hi - lo])
    return np.concatenate(parts, axis=0), res


def kernel(**inputs):
    x = np.asarray(inputs["x"], np.float32)
    eps = float(np.asarray(inputs["eps"]))
    b1 = np.asarray(inputs["beta1"], np.float32)
    b2 = np.asarray(inputs["beta2"], np.float32)
    b3 = np.asarray(inputs["beta3"], np.float32)
    si_list = [np.asarray(inputs[f"agg_scatter_index_{h}"]).astype(np.int64)
               for h in range(HOPS)]
    ni_list = [np.asarray(inputs[f"agg_node_index_{h}"]).astype(np.int64)
               for h in range(HOPS)]
    out, _ = run(x, eps, b1, b2, b3, si_list, ni_list, FULL_CFG)
    return out.astype(np.float32)

